# revision 12
# baseline (speedup 1.0000x reference)
"""Trainium2 Bass kernel for a custom GRU-like cell.

Reference (per row n; weights 256x256 applied x @ W.T + b):
    z        = sigmoid(x W_z^T + b_Wz + h U_z^T + b_Uz)
    r        = sigmoid(x W_r^T + b_Wr + h U_r^T + b_Ur)
    cand_in  = x W_h^T + b_Wh + (r*h) U_h^T + b_Uh + bias_h
    gate     = sigmoid(cand_in Lg^T + b_g)
    candidate= (cand_in Ll^T + b_l) * gate
    out      = z * candidate + (1 - z) * h

Strategy: data-parallel over rows across 8 cores.  All layout work is
hoisted to the host: x/h are uploaded transposed (feature-major) in both
bf16 and fp8-e4m3, so the PE does zero transposes.  The cand_in stage is
folded away algebraically: gate = sigmoid((Lg Wh) x + (Lg Uh)(r*h) + bg'),
candidate = ((Ll Wh) x + (Ll Uh)(r*h) + bl') * gate, with
bg' = Lg b_c + b_g, bl' = Ll b_c + b_l, b_c = b_Wh + b_Uh + bias_h.

Precision split (validated vs f32 reference, rel-err ~1.1e-2 < 2e-2):
  - z path: bf16 matmuls (z multiplies (candidate - h), the largest
    error amplifier, so it gets the accurate path)
  - r / gate / lin paths: fp8-e4m3 matmuls in DoubleRow perf mode
    (2 contraction chunks per instruction), weights pre-scaled x16 on
    the host, descaled by the activation scale=1/16
  - elementwise + output: bf16

Output y is written feature-major bf16 and transposed/upcast on host.
"""

import os

import numpy as np
import ml_dtypes

import concourse.bass as bass
import concourse.tile as tile
from concourse import bacc, mybir
from concourse import bass_utils

N_CORES = 8
D = 256
PART = 128
TILE_N = 512     # compute subtile (rows); one PSUM bank at f32
BLOCK_N = 1024   # DMA block (rows)

F32 = mybir.dt.float32
BF16 = mybir.dt.bfloat16
FP8 = mybir.dt.float8e4
AF = mybir.ActivationFunctionType
ALU = mybir.AluOpType
DR = mybir.MatmulPerfMode.DoubleRow
WSCALE = 16.0
NP_BF16 = ml_dtypes.bfloat16
NP_FP8 = ml_dtypes.float8_e4m3

# engine-assignment strategies (see emit_zr / emit_gl)
# NB: real TRN2 codegen rejects generic tensor ops on Pool/GPSIMD, so all
# elementwise work lives on DVE; ACT holds the three sigmoids.
RH_MODE = os.environ.get("GRU_RH", "dve8")  # dve8 | dve16 | pool8
CAND_STT = os.environ.get("GRU_CAND", "stt") == "stt"  # stt | dve
# lin m0 drained by ACT instead of DVE (engine balance); requires stt path
LIN_SPLIT = os.environ.get("GRU_LIN_SPLIT", "0") == "1" and CAND_STT


def _blocks_for(r_pad: int):
    assert r_pad % TILE_N == 0
    blocks = []
    n = r_pad
    while n >= BLOCK_N:
        blocks.append(BLOCK_N)
        n -= BLOCK_N
    if n:
        blocks.append(n)
    return blocks


def _build(r_pad: int, loop_reps: int = 1):
    blocks = _blocks_for(r_pad)

    nc = bacc.Bacc("TRN2", target_bir_lowering=False, debug=False)

    xt16_d = nc.dram_tensor("xt16", (D, r_pad), BF16, kind="ExternalInput")
    xt8_d = nc.dram_tensor("xt8", (D, r_pad), FP8, kind="ExternalInput")
    ht16_d = nc.dram_tensor("ht16", (D, r_pad), BF16, kind="ExternalInput")
    ht8_d = nc.dram_tensor("ht8", (D, r_pad), FP8, kind="ExternalInput")
    # fp8 weights [gemm(6), m, c_part, c_tile, out]: Wr, Ur, A1, A2, B1, B2
    w8_d = nc.dram_tensor("w8", (6, 2, PART, 2, PART), FP8, kind="ExternalInput")
    # bf16 z-path weights [w(2: Wz,Uz), m, c_tile, c_part, out]
    wz_d = nc.dram_tensor("wz16", (2, 2, 2, PART, PART), BF16, kind="ExternalInput")
    # biases [feat, 4]: z, r, gate, lin
    b_d = nc.dram_tensor("biases", (D, 4), F32, kind="ExternalInput")
    y_d = nc.dram_tensor("y", (D, r_pad), BF16, kind="ExternalOutput")

    xv16 = xt16_d.ap().rearrange("(c p) n -> p c n", p=PART)
    xv8 = xt8_d.ap().rearrange("(c p) n -> p c n", p=PART)
    hv16 = ht16_d.ap().rearrange("(c p) n -> p c n", p=PART)
    hv8 = ht8_d.ap().rearrange("(c p) n -> p c n", p=PART)
    yv = y_d.ap().rearrange("(c p) n -> p c n", p=PART)

    with tile.TileContext(nc) as tc:
        with (
            tc.tile_pool(name="const", bufs=1) as const,
            tc.tile_pool(name="io", bufs=2) as io,
            tc.tile_pool(name="act", bufs=3) as act,
            tc.tile_pool(name="ps", bufs=8, space="PSUM") as ps,
        ):
            w8_sb = const.tile([PART, 6, 2, 2, PART], FP8, tag="w8")
            wz_sb = const.tile([PART, 2, 2, 2, PART], BF16, tag="wz")
            b_sb = const.tile([PART, 2, 4], F32, tag="b")

            def emit_consts():
                nc.sync.dma_start(w8_sb[:], w8_d.ap().rearrange("g m p c o -> p g m c o"))
                nc.sync.dma_start(wz_sb[:], wz_d.ap().rearrange("w m c p o -> p w m c o"))
                nc.sync.dma_start(b_sb[:], b_d.ap().rearrange("(m p) j -> p m j", p=PART))

            starts = []
            n0 = 0
            for B in blocks:
                starts.append(n0)
                n0 += B
            nb = len(blocks)
            state = {}

            def emit_input(b):
                B = blocks[b]
                n0 = starts[b]
                x16 = io.tile([PART, 2, B], BF16, tag="x16")
                nc.sync.dma_start(x16[:], xv16[:, :, n0:n0 + B])
                h16 = io.tile([PART, 2, B], BF16, tag="h16")
                nc.sync.dma_start(h16[:], hv16[:, :, n0:n0 + B])
                x8 = io.tile([PART, 2, B], FP8, tag="x8")
                nc.sync.dma_start(x8[:], xv8[:, :, n0:n0 + B])
                h8 = io.tile([PART, 2, B], FP8, tag="h8")
                nc.sync.dma_start(h8[:], hv8[:, :, n0:n0 + B])
                y16 = io.tile([PART, 2, B], BF16, tag="y16")
                state[b] = dict(x16=x16, h16=h16, x8=x8, h8=h8, y16=y16)

            # stage 1 of subtile (b, t): r and z matmuls + activations + rh
            # (r first so rh is ready with a full subtile of slack for gl)
            def emit_zr(b, t):
                st = state[b]
                sl = slice(t * TILE_N, (t + 1) * TILE_N)
                x16, h16, x8, h8 = st["x16"], st["h16"], st["x8"], st["h8"]
                z_t = act.tile([PART, 2, TILE_N], BF16, tag="z_t")
                r_t = act.tile([PART, 2, TILE_N], BF16, tag="r_t")
                rh16 = act.tile([PART, 2, TILE_N], BF16, tag="rh16")
                rh8 = act.tile([PART, 2, TILE_N], FP8, tag="rh8")
                for m in range(2):
                    pr = ps.tile([PART, TILE_N], F32, tag="ps")
                    nc.tensor.matmul(pr[:], w8_sb[:, 0, m, :, :], x8[:, :, sl], start=True, stop=False, perf_mode=DR)
                    nc.tensor.matmul(pr[:], w8_sb[:, 1, m, :, :], h8[:, :, sl], start=False, stop=True, perf_mode=DR)
                    nc.scalar.activation(r_t[:, m, :], pr[:], AF.Sigmoid, bias=b_sb[:, m, 1:2], scale=1.0 / WSCALE)
                for m in range(2):
                    pz = ps.tile([PART, TILE_N], F32, tag="ps")
                    nc.tensor.matmul(pz[:], wz_sb[:, 0, m, 0, :], x16[:, 0, sl], start=True, stop=False)
                    nc.tensor.matmul(pz[:], wz_sb[:, 0, m, 1, :], x16[:, 1, sl], start=False, stop=False)
                    nc.tensor.matmul(pz[:], wz_sb[:, 1, m, 0, :], h16[:, 0, sl], start=False, stop=False)
                    nc.tensor.matmul(pz[:], wz_sb[:, 1, m, 1, :], h16[:, 1, sl], start=False, stop=True)
                    nc.scalar.activation(z_t[:, m, :], pz[:], AF.Sigmoid, bias=b_sb[:, m, 0:1])
                # rh = r*h -> fp8 for the gate/lin DoubleRow matmuls
                if RH_MODE == "pool8":
                    nc.gpsimd.scalar_tensor_tensor(rh8[:], r_t[:], 1.0, h16[:, :, sl], ALU.mult, ALU.mult)
                elif RH_MODE == "dve16":
                    # bf16 mul (2x 16-bit DVE mode) + 2x_2p cast
                    nc.vector.tensor_mul(rh16[:], r_t[:], h16[:, :, sl])
                    nc.vector.tensor_copy(rh8[:], rh16[:])
                else:  # dve8
                    nc.vector.tensor_mul(rh8[:], r_t[:], h16[:, :, sl])
                st[("zr", t)] = (z_t, r_t, rh8)

            # stage 2 of subtile (b, t): gate/lin matmuls + combine + y
            def emit_gl(b, t):
                st = state[b]
                sl = slice(t * TILE_N, (t + 1) * TILE_N)
                x8, h16, y16 = st["x8"], st["h16"], st["y16"]
                z_t, r_t, rh8 = st.pop(("zr", t))
                g_t = act.tile([PART, 2, TILE_N], BF16, tag="g_t")
                l_t = act.tile([PART, 2, TILE_N], BF16, tag="l_t")
                for m in range(2):
                    pg = ps.tile([PART, TILE_N], F32, tag="ps")
                    nc.tensor.matmul(pg[:], w8_sb[:, 2, m, :, :], x8[:, :, sl], start=True, stop=False, perf_mode=DR)
                    nc.tensor.matmul(pg[:], w8_sb[:, 3, m, :, :], rh8[:], start=False, stop=True, perf_mode=DR)
                    nc.scalar.activation(g_t[:, m, :], pg[:], AF.Sigmoid, bias=b_sb[:, m, 2:3], scale=1.0 / WSCALE)
                    pl = ps.tile([PART, TILE_N], F32, tag="ps")
                    nc.tensor.matmul(pl[:], w8_sb[:, 4, m, :, :], x8[:, :, sl], start=True, stop=False, perf_mode=DR)
                    nc.tensor.matmul(pl[:], w8_sb[:, 5, m, :, :], rh8[:], start=False, stop=True, perf_mode=DR)
                    if LIN_SPLIT and m == 0:
                        # balance: lin m0 drains PSUM via ACT (identity+bias),
                        # its gate-multiply folds into the m0 STT below
                        nc.scalar.activation(l_t[:, m, :], pl[:], AF.Identity,
                                             bias=b_sb[:, m, 3:4], scale=1.0 / WSCALE)
                        nc.vector.scalar_tensor_tensor(
                            l_t[:, m, :], l_t[:, m, :], 1.0, g_t[:, m, :], ALU.mult, ALU.mult)
                    elif CAND_STT:
                        # cand = (pl/16 + bl)*g = (pl/16)*g + bl*g as two fused
                        # STT ops; the second is all-bf16 SBUF (fast DVE mode)
                        nc.vector.scalar_tensor_tensor(
                            l_t[:, m, :], pl[:], 1.0 / WSCALE, g_t[:, m, :], ALU.mult, ALU.mult)
                        nc.vector.scalar_tensor_tensor(
                            l_t[:, m, :], g_t[:, m, :], b_sb[:, m, 3:4], l_t[:, m, :], ALU.mult, ALU.add)
                    else:
                        nc.vector.tensor_scalar(l_t[:, m, :], pl[:], 1.0 / WSCALE, b_sb[:, m, 3:4], ALU.mult, ALU.add)
                if not CAND_STT:
                    nc.vector.tensor_mul(l_t[:], l_t[:], g_t[:])
                # y = z*(cand - h) + h, in place over l_t
                nc.vector.tensor_sub(l_t[:], l_t[:], h16[:, :, sl])
                nc.vector.tensor_mul(l_t[:], l_t[:], z_t[:])
                nc.vector.tensor_add(y16[:, :, sl], l_t[:], h16[:, :, sl])

            def emit_output(b):
                B = blocks[b]
                n0 = starts[b]
                nc.sync.dma_start(yv[:, :, n0:n0 + B], state[b]["y16"][:])
                del state[b]

            # global subtile schedule: (block, t) pairs; PE stays busy by
            # running gate/lin of subtile s-1 between z/r of s and s+1
            subtiles = []
            for b, B in enumerate(blocks):
                for t in range(B // TILE_N):
                    subtiles.append((b, t))
            ns = len(subtiles)

            def whole_pass():
                emit_input(0)
                done_blocks = 0
                for s in range(ns):
                    b, t = subtiles[s]
                    if t == 0 and b + 1 < nb:
                        emit_input(b + 1)
                    emit_zr(b, t)
                    if s > 0:
                        pb, pt = subtiles[s - 1]
                        emit_gl(pb, pt)
                        if pb != b:
                            emit_output(pb)
                b, t = subtiles[ns - 1]
                emit_gl(b, t)
                emit_output(b)
                state.clear()

            loop_reps = int(os.environ.get("GRU_LOOP_REPS", str(loop_reps)))
            if loop_reps > 1:
                emit_consts()
                with tc.For_i(0, loop_reps, 1):
                    whole_pass()
            else:
                emit_consts()
                whole_pass()

    nc.compile()
    return nc


_NC_CACHE: dict[int, object] = {}


def _get_nc(r_pad: int):
    if r_pad not in _NC_CACHE:
        _NC_CACHE[r_pad] = _build(r_pad)
    return _NC_CACHE[r_pad]


def _prep_weights(W_z_w, W_z_b, U_z_w, U_z_b,
                  W_r_w, W_r_b, U_r_w, U_r_b,
                  W_h_w, W_h_b, U_h_w, U_h_b,
                  lin_gate_w, lin_gate_b, lin_linear_w, lin_linear_b,
                  bias_h):
    f8 = lambda a: np.asarray(a, np.float64)
    Wz, Uz, Wr, Ur, Wh, Uh, Lg, Ll = map(
        f8, (W_z_w, U_z_w, W_r_w, U_r_w, W_h_w, U_h_w, lin_gate_w, lin_linear_w))
    A1 = Lg @ Wh
    A2 = Lg @ Uh
    B1 = Ll @ Wh
    B2 = Ll @ Uh

    def fp8_st(Wm):  # (out,in) -> DoubleRow stationary [m, c_part, c_tile, out]
        Wp = np.ascontiguousarray(Wm.T)  # [in, out]
        q = (np.asarray(Wp * WSCALE, np.float32)).astype(NP_FP8)
        return q.reshape(2, PART, 2, PART).transpose(2, 1, 0, 3)

    def bf16_st(Wm):  # (out,in) -> [m, c_tile, c_part, out]
        Wp = np.ascontiguousarray(Wm.T)
        q = np.asarray(Wp, np.float32).astype(NP_BF16)
        return q.reshape(2, PART, 2, PART).transpose(2, 0, 1, 3)

    w8 = np.stack([fp8_st(M) for M in (Wr, Ur, A1, A2, B1, B2)])
    wz16 = np.stack([bf16_st(M) for M in (Wz, Uz)])

    b_c = f8(W_h_b) + f8(U_h_b) + f8(bias_h)
    biases = np.stack([
        f8(W_z_b) + f8(U_z_b),
        f8(W_r_b) + f8(U_r_b),
        Lg @ b_c + f8(lin_gate_b),
        Ll @ b_c + f8(lin_linear_b),
    ], axis=1).astype(np.float32)
    return w8, wz16, biases


def kernel(x, h,
           W_z_w, W_z_b, U_z_w, U_z_b,
           W_r_w, W_r_b, U_r_w, U_r_b,
           W_h_w, W_h_b, U_h_w, U_h_b,
           lin_gate_w, lin_gate_b, lin_linear_w, lin_linear_b,
           bias_h):
    x = np.asarray(x, dtype=np.float32)
    h = np.asarray(h, dtype=np.float32)
    n_rows = x.shape[0]

    w8, wz16, biases = _prep_weights(
        W_z_w, W_z_b, U_z_w, U_z_b, W_r_w, W_r_b, U_r_w, U_r_b,
        W_h_w, W_h_b, U_h_w, U_h_b,
        lin_gate_w, lin_gate_b, lin_linear_w, lin_linear_b, bias_h)

    per = (n_rows + N_CORES - 1) // N_CORES
    bounds = [(c * per, min((c + 1) * per, n_rows)) for c in range(N_CORES)]
    r_max = max(e - s for s, e in bounds)
    r_pad = ((r_max + TILE_N - 1) // TILE_N) * TILE_N

    in_maps = []
    for s, e in bounds:
        n = e - s
        xt16 = np.zeros((D, r_pad), NP_BF16)
        ht16 = np.zeros((D, r_pad), NP_BF16)
        xt8 = np.zeros((D, r_pad), NP_FP8)
        ht8 = np.zeros((D, r_pad), NP_FP8)
        xs = x[s:e]
        hs = h[s:e]
        xt16[:, :n] = np.ascontiguousarray(xs.astype(NP_BF16).T)
        ht16[:, :n] = np.ascontiguousarray(hs.astype(NP_BF16).T)
        xt8[:, :n] = np.ascontiguousarray(xs.astype(NP_FP8).T)
        ht8[:, :n] = np.ascontiguousarray(hs.astype(NP_FP8).T)
        in_maps.append({"xt16": xt16, "xt8": xt8, "ht16": ht16, "ht8": ht8,
                        "w8": w8, "wz16": wz16, "biases": biases})

    nc = _get_nc(r_pad)
    res = bass_utils.run_bass_kernel_spmd(nc, in_maps, core_ids=list(range(N_CORES)))

    out = np.empty((n_rows, D), np.float32)
    for c, (s, e) in enumerate(bounds):
        yt = res.results[c]["y"]  # [D, r_pad] bf16
        out[s:e] = yt[:, : e - s].T.astype(np.float32)
    return out
